# revision 19
# baseline (speedup 1.0000x reference)
"""DARTS-cell (moe_routing) Trainium2 kernel — sparse per-core slot grid.

Data-parallel over batch B=32 across 8 cores (4 samples/core). Top-2-of-8
routing: only ~1.77 of 7 branches are active per (m, sample). The program
is specialized at runtime to a padded per-(step, branch-type) slot grid
sized by the max count over cores (SPMD: one program, per-core DATA
selects the work). Slots are shared across the core's 4 samples:

- slot inputs: gpsimd.ap_gather from the state stack [128, 4*6, 1024]
  (bf16), indices host-packed per core,
- slot weights/alphas: host-gathered per-slot DRAM tables,
- slot outputs: psum -> bf16 outbuf -> gpsimd.scatter_add into the
  data-dependent target state (16 distinct chunks per scatter, so no
  duplicate-index ambiguity),
- dummy (padding) slots run with zero weights/alphas and add zeros.

Conv branch forms are chosen per type at plan time (PE tap-fused matmuls
vs bf16 DVE dw-chains + pw matmul) to balance engine load. All branches
share one zero-margin z-buffer geometry [128,40,40], interior [4,36).
BN (eval, affine=False) folded into weights/maps on host.
"""

import sys

sys.path.insert(0, "/opt/trn_rl_repo")

import numpy as np
from concourse import bacc, mybir, tile
from concourse.bass_utils import run_bass_kernel_spmd

STEPS = 4
N_MIX = 14
OFFSETS = [0, 2, 5, 9]
B, C_IN, C, H, W = 32, 512, 128, 32, 32
HW = H * W
N_CORES = 8
BL = B // N_CORES
BN_SCALE = float(1.0 / np.sqrt(1.0 + 1e-5))

F32 = mybir.dt.float32
F32R = mybir.dt.float32r
BF16 = mybir.dt.bfloat16
I16 = mybir.dt.int16
ALU = mybir.AluOpType
ACTF = mybir.ActivationFunctionType

O_MAX, O_AVG, O_SKIP, O_SEP3, O_SEP5, O_DIL3, O_DIL5 = 1, 2, 3, 4, 5, 6, 7
CONV_OPS = (O_SEP5, O_SEP3, O_DIL5, O_DIL3)
CHEAP_OPS = (O_MAX, O_AVG, O_SKIP)
TYPE_NAME = {O_SEP3: "s3", O_SEP5: "s5", O_DIL3: "d3", O_DIL5: "d5",
             O_MAX: "max", O_AVG: "avg", O_SKIP: "skip"}
CONV_GEO = {O_SEP3: (3, 1, 1), O_SEP5: (5, 2, 1),
            O_DIL3: (3, 2, 2), O_DIL5: (5, 4, 2)}
TWO_STAGE = {O_SEP3, O_SEP5}

# unit costs (us) for form selection / balancing (measured-calibrated)
UC_PE = {O_SEP5: 27.8, O_DIL5: 13.9, O_SEP3: 10.0, O_DIL3: 5.0}
UC_DVE = {O_SEP5: 27.7, O_DIL5: 13.9, O_SEP3: 10.0, O_DIL3: 5.0}  # bf16 2x
UC_PW = {O_SEP5: 2.2, O_DIL5: 1.1, O_SEP3: 2.2, O_DIL3: 1.1}
UC_RELU = {O_SEP5: 1.7, O_DIL5: 0.85, O_SEP3: 1.7, O_DIL3: 0.85}


def _host_alphas(gates, top):
    g = np.asarray(gates).astype(np.float64)
    idx = np.argsort(-g, axis=-1, kind="stable")[..., :top]
    mask = np.zeros(g.shape, bool)
    np.put_along_axis(mask, idx, True, axis=-1)
    gm = np.where(mask, g, -np.inf)
    gm -= gm.max(axis=-1, keepdims=True)
    e = np.exp(gm)
    p = e / e.sum(axis=-1, keepdims=True)
    return p.astype(np.float32)


# ---------------------------------------------------------------- planning

def _grid(act, assign):
    """K[(s,o)] = max over cores of per-step per-type item count."""
    K = {}
    for s in range(STEPS):
        for o in CONV_OPS + CHEAP_OPS:
            mx = 0
            for core in range(N_CORES):
                c = sum(int(act[OFFSETS[s] + j, smp, o])
                        for smp in assign[core] for j in range(2 + s))
                if c > mx:
                    mx = c
            K[(s, o)] = mx
    return K


def _grid_cost(act, assign):
    K = _grid(act, assign)
    pe = dve = 0.0
    for (s, o), k in K.items():
        if o in UC_PE:
            pe += k * UC_PE[o]
        else:
            dve += k * 3.0
    return max(pe, dve) + 0.15 * (pe + dve), K


def _optimize_assignment(act, iters=4000, seed=0):
    rng = np.random.default_rng(seed)
    w = np.zeros(B)
    for m in range(N_MIX):
        for b in range(B):
            w[b] += sum(UC_PE[o] for o in CONV_OPS if act[m, b, o])
    order = np.argsort(-w)
    loads = [0.0] * N_CORES
    assign = [[] for _ in range(N_CORES)]
    for b in order:
        c = min((i for i in range(N_CORES) if len(assign[i]) < BL),
                key=lambda i: loads[i])
        assign[c].append(int(b))
        loads[c] += w[b]
    cur = [list(a) for a in assign]
    cur_cost, _ = _grid_cost(act, cur)
    best, best_cost = [list(a) for a in cur], cur_cost
    for it in range(iters):
        c1, c2 = rng.integers(0, N_CORES, 2)
        s1, s2 = rng.integers(0, BL, 2)
        if c1 == c2:
            continue
        cur[c1][s1], cur[c2][s2] = cur[c2][s2], cur[c1][s1]
        cost, _ = _grid_cost(act, cur)
        if cost <= cur_cost:
            cur_cost = cost
            if cost < best_cost:
                best_cost, best = cost, [list(a) for a in cur]
        else:
            cur[c1][s1], cur[c2][s2] = cur[c2][s2], cur[c1][s1]
    return best, _grid(act, best)


def _pick_forms(K):
    """Choose pe/dve form per conv type + avg engine to balance loads."""
    n = {o: sum(K[(s, o)] for s in range(STEPS)) for o in CONV_OPS + CHEAP_OPS}
    n_conv = sum(n[o] for o in CONV_OPS)
    n_cheap = sum(n[o] for o in CHEAP_OPS)
    n_slots = n_conv + n_cheap
    best = None
    import itertools
    for combo in itertools.product(("pe", "dve"), repeat=4):
        forms = dict(zip(CONV_OPS, combo))
        for avg_eng in ("gpsimd", "dve"):
            pe = 34.0 + sum(n[o] * UC_PE[o] for o in CONV_OPS
                            if forms[o] == "pe")
            pe += sum(n[o] * UC_PW[o] for o in CONV_OPS if forms[o] == "dve")
            dve = sum(n[o] * UC_DVE[o] for o in CONV_OPS if forms[o] == "dve")
            dve += n[O_MAX] * 2.8
            dve += n[O_AVG] * (3.4 if avg_eng == "dve" else 0.6)
            gps = n_slots / 2 * 2.85 + n_slots * 1.42 + 30.0
            gps += n[O_AVG] * (10.8 if avg_eng == "gpsimd" else 0.0)
            sca = 60.0 + sum(n[o] * UC_RELU[o] for o in CONV_OPS)
            sca += n_conv * 0.85 + n_cheap * 0.85
            span = max(pe, dve, gps, sca) + 0.10 * (pe + dve + gps + sca)
            if best is None or span < best[0]:
                best = (span, forms, avg_eng, dict(pe=pe, dve=dve, gps=gps,
                                                   sca=sca))
    return best[1], best[2], best[3]


def build_plan(gates, top):
    p = _host_alphas(gates, top)
    act = p > 0
    assign, K = _optimize_assignment(act)
    forms, avg_eng, loads = _pick_forms(K)

    # per-step emission schedule: conv types round-robin, then cheap
    sched = {}
    for s in range(STEPS):
        convs = []
        rem = {o: K[(s, o)] for o in CONV_OPS}
        while any(rem.values()):
            for o in CONV_OPS:
                if rem[o]:
                    convs.append(o)
                    rem[o] -= 1
        cheaps = []
        for o in CHEAP_OPS:
            cheaps += [o] * K[(s, o)]
        sched[s] = (tuple(convs), tuple(cheaps))

    n_stage = n_pw = n_dve = n_conv = n_cheap = n_wave = 0
    for s in range(STEPS):
        convs, cheaps = sched[s]
        for o in convs:
            if forms[o] == "pe":
                n_stage += 2 if o in TWO_STAGE else 1
            else:
                n_pw += 2 if o in TWO_STAGE else 1
                n_dve += 1
        n_conv += len(convs)
        n_cheap += len(cheaps)
        n_wave += (len(convs) + len(cheaps) + 1) // 2

    key = (tuple(sorted(K.items())), tuple(sorted(forms.items())), avg_eng)
    return dict(p=p, act=act, assign=assign, K=K, sched=sched, forms=forms,
                avg_eng=avg_eng, loads=loads, key=key,
                n=dict(stage=max(n_stage, 1), pw=max(n_pw, 1),
                       dve=max(n_dve, 1), conv=max(n_conv, 1),
                       cheap=max(n_cheap, 1), wave=max(n_wave, 1),
                       slots=max(n_conv + n_cheap, 1)))


# ---------------------------------------------------------------- program

def build_program(plan, n_cores=N_CORES):
    sched, forms, avg_eng, n = (plan["sched"], plan["forms"],
                                plan["avg_eng"], plan["n"])
    nc = bacc.Bacc("TRN2", target_bir_lowering=False, debug=False,
                   num_devices=n_cores)

    x0_d = nc.dram_tensor("x0", [BL, 4, 128, HW], F32, kind="ExternalInput").ap()
    x1_d = nc.dram_tensor("x1", [BL, 4, 128, HW], F32, kind="ExternalInput").ap()
    prew_d = nc.dram_tensor("prew", [128, 2, 4, 128], F32R, kind="ExternalInput").ap()
    fw_d = nc.dram_tensor("fw", [128, n["stage"], 25, 128], BF16, kind="ExternalInput").ap()
    pw_d = nc.dram_tensor("pw", [128, n["pw"], 128], BF16, kind="ExternalInput").ap()
    dwt_d = nc.dram_tensor("dwt", [128, n["dve"], 50], F32, kind="ExternalInput").ap()
    alf_d = nc.dram_tensor("alf", [128, n["conv"]], F32, kind="ExternalInput").ap()
    alfc_d = nc.dram_tensor("alfc", [128, n["cheap"]], F32, kind="ExternalInput").ap()
    idx_d = nc.dram_tensor("idx", [128, n["wave"]], I16, kind="ExternalInput").ap()
    six_d = nc.dram_tensor("six", [128, n["slots"]], I16, kind="ExternalInput").ap()
    rmap_d = nc.dram_tensor("rmap", [128, 32, 32], F32, kind="ExternalInput").ap()
    out_d = nc.dram_tensor("out", [BL, 4, 128, HW], F32, kind="ExternalOutput").ap()

    with tile.TileContext(nc) as tc:
        with (
            tc.tile_pool(name="const", bufs=1) as cpool,
            tc.tile_pool(name="work", bufs=1) as wpool,
            tc.tile_pool(name="xs", bufs=2) as xpool,
            tc.tile_pool(name="stg", bufs=3) as spool,
            tc.tile_pool(name="dwa", bufs=3) as dpool,
            tc.tile_pool(name="pwb", bufs=2) as pwpool,
            tc.tile_pool(name="fw", bufs=2) as fwpool,
            tc.tile_pool(name="ob", bufs=4) as obpool,
            tc.tile_pool(name="ost", bufs=2) as opool,
            tc.tile_pool(name="ps_state", bufs=2, space="PSUM") as pspool,
            tc.tile_pool(name="ps_scr", bufs=2, space="PSUM") as scrpool,
        ):
            prew = cpool.tile([128, 2, 4, 128], F32R, tag="prew")
            dwt = cpool.tile([128, n["dve"], 50], F32, tag="dwt")
            alf = cpool.tile([128, n["conv"]], F32, tag="alf")
            alfc = cpool.tile([128, n["cheap"]], F32, tag="alfc")
            idx = cpool.tile([128, n["wave"]], I16, tag="idx")
            six = cpool.tile([128, n["slots"]], I16, tag="six")
            rmap = cpool.tile([128, 32, 32], F32, tag="rmap")
            for t, d in ((prew, prew_d), (dwt, dwt_d), (alf, alf_d),
                         (alfc, alfc_d), (idx, idx_d), (six, six_d),
                         (rmap, rmap_d)):
                nc.sync.dma_start(t[:], d)

            # state stack: 4 samples x 6 states, bf16
            states = wpool.tile([128, 24, 1024], BF16, tag="states")

            # pool scratch
            xpmax = wpool.tile([128, 34, 34], BF16, tag="xpmax")
            rmpad = wpool.tile([128, 34, 32], BF16, tag="rmpad")
            ptmp0 = wpool.tile([128, 32, 32], BF16, tag="ptmp0")
            xpsum = wpool.tile([128, 34, 34], F32, tag="xpsum")
            rspad = wpool.tile([128, 34, 32], F32, tag="rspad")
            ptmp1 = wpool.tile([128, 32, 32], F32, tag="ptmp1")
            nc.gpsimd.memset(xpmax[:], -1e30)
            nc.gpsimd.memset(rmpad[:], -1e30)
            nc.gpsimd.memset(xpsum[:], 0.0)
            nc.gpsimd.memset(rspad[:], 0.0)
            # states 2..5 of each sample start at zero (scatter accumulates)
            for bs in range(BL):
                nc.vector.memset(states[:, 6 * bs + 2:6 * bs + 6, :], 0.0)

            zbufs = [wpool.tile([128, 40, 40], BF16, tag=f"z{i}",
                                name=f"z{i}") for i in range(4)]
            for z in zbufs:
                nc.gpsimd.memset(z[:], 0.0)
            zctr = [0]

            def flat(ap3):
                return ap3.rearrange("p a b -> p (a b)")

            def relu_into_z(src_ap, scale):
                z = zbufs[zctr[0] % len(zbufs)]
                zctr[0] += 1
                nc.scalar.activation(z[:, 4:36, 4:36], src_ap, ACTF.Relu,
                                     scale=scale)
                return z

            def mm_chunks(psum3, lhsT, rhs3, flags):
                s0, e0, s1, e1 = flags
                nc.tensor.matmul(psum3[:, 0:16, :], lhsT, rhs3[:, 0:16, :],
                                 start=s0, stop=e0)
                nc.tensor.matmul(psum3[:, 16:32, :], lhsT, rhs3[:, 16:32, :],
                                 start=s1, stop=e1)

            def dw_chain(z, dslot, tap0, k, pad, stride):
                dwacc = dpool.tile([128, 32, 32], BF16, tag="dwacc")
                first = True
                for ky in range(k):
                    for kx in range(k):
                        t = tap0 + ky * k + kx
                        y0 = 4 - pad + stride * ky
                        x0 = 4 - pad + stride * kx
                        view = z[:, y0:y0 + 32, x0:x0 + 32]
                        sc = dwt[:, dslot, t:t + 1]
                        if first:
                            nc.vector.tensor_scalar_mul(dwacc[:], view, sc)
                            first = False
                        else:
                            nc.vector.scalar_tensor_tensor(
                                dwacc[:], view, sc, dwacc[:],
                                op0=ALU.mult, op1=ALU.add)
                return dwacc

            def fused_stage(stage_i, z, k, pad, stride, psum3):
                taps = k * k
                half = (taps + 1) // 2
                for (a, e) in ((0, half), (half, taps)):
                    fwt = fwpool.tile([128, 13, 128], BF16, tag="fw")
                    nc.sync.dma_start(fwt[:, 0:e - a, :],
                                      fw_d[:, stage_i, a:e, :])
                    for t in range(a, e):
                        ky, kx = divmod(t, k)
                        y0 = 4 - pad + stride * ky
                        x0 = 4 - pad + stride * kx
                        for h2 in range(2):
                            nc.tensor.matmul(
                                psum3[:, 16 * h2:16 * h2 + 16, :],
                                fwt[:, t - a, :],
                                z[:, y0 + 16 * h2:y0 + 16 * h2 + 16,
                                  x0:x0 + 32],
                                start=(t == 0), stop=(t == taps - 1))
                return psum3

            def stream_pw(pw_i):
                t = pwpool.tile([128, 1, 128], BF16, tag="pwb")
                nc.sync.dma_start(t[:], pw_d[:, pw_i:pw_i + 1, :])
                return t[:, 0, :]

            def conv_slot(o, x_ap, cs, ctr):
                """Emit conv slot; returns stp psum [128,32,32] result."""
                k, pad, stride = CONV_GEO[o]
                a_ap = alf[:, cs:cs + 1]
                stp = pspool.tile([128, 32, 32], F32, tag="stp")
                if forms[o] == "pe":
                    z1 = relu_into_z(x_ap, a_ap)
                    if o in TWO_STAGE:
                        scr = scrpool.tile([128, 32, 32], F32, tag="scr")
                        fused_stage(ctr["stage"], z1, k, pad, stride, scr)
                        ctr["stage"] += 1
                        z2 = relu_into_z(scr[:], 1.0)
                        fused_stage(ctr["stage"], z2, k, pad, stride, stp)
                        ctr["stage"] += 1
                    else:
                        fused_stage(ctr["stage"], z1, k, pad, stride, stp)
                        ctr["stage"] += 1
                else:
                    z1 = relu_into_z(x_ap, a_ap)
                    dwacc = dw_chain(z1, ctr["dve"], 0, k, pad, stride)
                    if o in TWO_STAGE:
                        scr = scrpool.tile([128, 32, 32], F32, tag="scr")
                        mm_chunks(scr, stream_pw(ctr["pw"]), dwacc,
                                  (True, True, True, True))
                        ctr["pw"] += 1
                        z2 = relu_into_z(scr[:], 1.0)
                        dwacc2 = dw_chain(z2, ctr["dve"], 25, k, pad, stride)
                        mm_chunks(stp, stream_pw(ctr["pw"]), dwacc2,
                                  (True, True, True, True))
                        ctr["pw"] += 1
                    else:
                        mm_chunks(stp, stream_pw(ctr["pw"]), dwacc,
                                  (True, True, True, True))
                        ctr["pw"] += 1
                    ctr["dve"] += 1
                return stp

            def cheap_slot(o, x_ap, cc_i, ob):
                """Compute cheap op into ob tile [128,32,32] (alpha-scaled)."""
                sc = alfc[:, cc_i:cc_i + 1]
                dst = ob[:]
                if o == O_SKIP:
                    nc.scalar.activation(dst, x_ap, ACTF.Copy, scale=sc)
                elif o == O_MAX:
                    nc.scalar.copy(xpmax[:, 1:33, 1:33], x_ap)
                    t = ptmp0
                    nc.vector.tensor_max(t[:], xpmax[:, 1:33, 0:32],
                                         xpmax[:, 1:33, 1:33])
                    nc.vector.tensor_max(rmpad[:, 1:33, :], t[:],
                                         xpmax[:, 1:33, 2:34])
                    nc.vector.tensor_max(t[:], rmpad[:, 0:32, :],
                                         rmpad[:, 1:33, :])
                    nc.vector.tensor_max(t[:], t[:], rmpad[:, 2:34, :])
                    nc.scalar.activation(dst, t[:], ACTF.Copy, scale=sc)
                else:  # O_AVG
                    eng = nc.gpsimd if avg_eng == "gpsimd" else nc.vector
                    nc.scalar.copy(xpsum[:, 1:33, 1:33], x_ap)
                    t = ptmp1
                    eng.tensor_add(t[:], xpsum[:, 1:33, 0:32],
                                   xpsum[:, 1:33, 1:33])
                    eng.tensor_add(rspad[:, 1:33, :], t[:],
                                   xpsum[:, 1:33, 2:34])
                    eng.tensor_add(t[:], rspad[:, 0:32, :],
                                   rspad[:, 1:33, :])
                    eng.tensor_add(t[:], t[:], rspad[:, 2:34, :])
                    eng.tensor_mul(t[:], t[:], rmap[:])
                    nc.scalar.activation(dst, t[:], ACTF.Copy, scale=sc)

            # ---- preprocess ----
            for bs in range(BL):
                for inp, xd in ((0, x0_d), (1, x1_d)):
                    scr = scrpool.tile([128, 32, 32], F32, tag="scr")
                    for kc in range(4):
                        xb = xpool.tile([128, HW], F32, tag="xb")
                        nc.sync.dma_start(xb[:], xd[bs, kc])
                        xr = xpool.tile([128, HW], F32R, tag="xr")
                        nc.scalar.activation(xr[:], xb[:], ACTF.Relu)
                        for h in range(2):
                            nc.tensor.matmul(
                                scr[:, 16 * h:16 * (h + 1), :],
                                prew[:, inp, kc, :],
                                xr[:, 512 * h:512 * (h + 1)].rearrange(
                                    "p (a c) -> p a c", a=16),
                                start=(kc == 0), stop=(kc == 3))
                    nc.scalar.copy(states[:, 6 * bs + inp, :].rearrange(
                        "p (h w) -> p h w", h=32), scr[:])

            # ---- steps ----
            ctr = dict(stage=0, pw=0, dve=0)
            n_slot_c = [0]
            n_conv_c = n_cheap_c = n_wave_c = 0
            for s in range(STEPS):
                convs, cheaps = sched[s]
                n_slots = len(convs) + len(cheaps)
                n_waves = (n_slots + 1) // 2
                stgs = {}

                def slot_x(i):
                    wv = i // 2
                    if wv not in stgs:
                        stg = spool.tile([128, 16, 128], BF16, tag="stg")
                        nc.gpsimd.ap_gather(
                            flat(stg[:]), flat(states[:]),
                            idx[:, n_wave_c + wv:n_wave_c + wv + 1],
                            channels=128, num_elems=192, d=128, num_idxs=16)
                        stgs[wv] = stg
                    stg = stgs[wv]
                    half = stg[:, 8 * (i % 2):8 * (i % 2) + 8, :]
                    return flat(half).rearrange("p (h w) -> p h w", h=32)

                def scatter(ob):
                    nc.gpsimd.scatter_add(
                        states[:].rearrange("p a b -> p (a b)")
                        .rearrange("p (a b) -> p a b", b=64),
                        six[:, n_slot_c[0]:n_slot_c[0] + 1],
                        flat(ob[:]).rearrange("p (a b) -> p a b", b=64),
                        channels=128, num_elems=384, d=64, num_idxs=16)
                    n_slot_c[0] += 1

                for i, o in enumerate(convs):
                    stp = conv_slot(o, slot_x(i), n_conv_c + i, ctr)
                    ob = obpool.tile([128, 32, 32], BF16, tag="ob")
                    nc.scalar.copy(ob[:], stp[:])
                    scatter(ob)
                for i, o in enumerate(cheaps):
                    ob = obpool.tile([128, 32, 32], BF16, tag="ob")
                    cheap_slot(o, slot_x(len(convs) + i), n_cheap_c + i, ob)
                    scatter(ob)
                n_conv_c += len(convs)
                n_cheap_c += len(cheaps)
                n_wave_c += n_waves

            # ---- output: states 2..5 per sample -> f32 DMA ----
            for bs in range(BL):
                for i in range(4):
                    ost = opool.tile([128, 1024], F32, tag="ost")
                    nc.scalar.copy(ost[:], states[:, 6 * bs + 2 + i, :])
                    nc.sync.dma_start(out_d[bs, i], ost[:])

    nc.compile()
    return nc


# ---------------------------------------------------------------- host data

def host_prepare(inputs):
    s0, s1 = np.asarray(inputs["s0"]), np.asarray(inputs["s1"])
    gates = np.asarray(inputs["gates"])
    top = int(inputs["top"])
    plan = build_plan(gates, top)
    p, assign, sched, forms, n = (plan["p"], plan["assign"], plan["sched"],
                                  plan["forms"], plan["n"])

    prew = np.empty((128, 2, 4, 128), np.float32)
    for inp, wname in ((0, "pre0_w"), (1, "pre1_w")):
        wmat = np.asarray(inputs[wname]) * BN_SCALE
        for kc in range(4):
            prew[:, inp, kc, :] = wmat[:, 128 * kc:128 * (kc + 1)].T

    FUSE_KEYS = {O_SEP5: (("sep5_pw1", "sep5_dw1"), ("sep5_pw2", "sep5_dw2")),
                 O_DIL5: (("dil5_pw", "dil5_dw"),),
                 O_SEP3: (("sep3_pw1", "sep3_dw1"), ("sep3_pw2", "sep3_dw2")),
                 O_DIL3: (("dil3_pw", "dil3_dw"),)}

    def fuse(pw_key, dw_key, m, k):
        pwm = np.asarray(inputs[pw_key])[m].astype(np.float32) * BN_SCALE
        dwm = np.asarray(inputs[dw_key])[m].astype(np.float32).reshape(C, k * k)
        return pwm.T[:, None, :] * dwm[:, :, None]  # [ci, k*k, co]

    cnt = np.zeros((32, 32), np.float32)
    for dy in (-1, 0, 1):
        for dx in (-1, 0, 1):
            cnt[max(0, dy):32 - max(0, -dy),
                max(0, dx):32 - max(0, -dx)] += 1
    rmap = np.broadcast_to((BN_SCALE / cnt).astype(np.float32),
                           (128, 32, 32)).copy()

    act = plan["act"]
    in_maps = []
    for core in range(N_CORES):
        samples = assign[core]
        fw = np.zeros((128, n["stage"], 25, 128), np.float32)
        pw = np.zeros((128, n["pw"], 128), np.float32)
        dwt = np.zeros((128, n["dve"], 50), np.float32)
        alf_t = np.zeros((n["conv"],), np.float32)
        alfc_t = np.zeros((n["cheap"],), np.float32)
        idx_t = np.zeros((128, n["wave"]), np.int16)
        six_t = np.zeros((128, n["slots"]), np.int16)
        ns = dict(stage=0, pw=0, dve=0, conv=0, cheap=0, wave=0, slot=0)

        for s in range(STEPS):
            convs, cheaps = sched[s]
            items = {o: [] for o in CONV_OPS + CHEAP_OPS}
            for bs in range(BL):
                smp = samples[bs]
                for j in range(2 + s):
                    m = OFFSETS[s] + j
                    for o in CONV_OPS + CHEAP_OPS:
                        if act[m, smp, o]:
                            items[o].append((m, j, bs))
            used = {o: 0 for o in items}
            slot_src = []   # gather chunk base per slot (j-state of its bs)
            slot_tgt = []   # scatter target state index per slot
            for o in convs:
                if used[o] < len(items[o]):
                    m, j, bs = items[o][used[o]]
                    used[o] += 1
                    a = float(p[m, samples[bs], o])
                else:
                    m, j, bs, a = None, 0, 0, 0.0
                slot_src.append(6 * bs + j)
                slot_tgt.append(6 * bs + 2 + s)
                alf_t[ns["conv"]] = a
                k, _, _ = CONV_GEO[o]
                if forms[o] == "pe":
                    for st_i, (pwk, dwk) in enumerate(FUSE_KEYS[o]):
                        if m is not None:
                            fw[:, ns["stage"], 0:k * k] = fuse(pwk, dwk, m, k)
                        ns["stage"] += 1
                else:
                    if m is not None:
                        for st_i, (pwk, dwk) in enumerate(FUSE_KEYS[o]):
                            dwm = np.asarray(inputs[dwk])[m].reshape(C, k * k)
                            dwt[:, ns["dve"], 25 * st_i:25 * st_i + k * k] = dwm
                            pw[:, ns["pw"] + st_i] = (
                                np.asarray(inputs[pwk])[m].T * BN_SCALE)
                    ns["pw"] += 2 if o in TWO_STAGE else 1
                    ns["dve"] += 1
                ns["conv"] += 1
            for o in cheaps:
                if used[o] < len(items[o]):
                    m, j, bs = items[o][used[o]]
                    used[o] += 1
                    a = float(p[m, samples[bs], o])
                else:
                    m, j, bs, a = None, 0, 0, 0.0
                slot_src.append(6 * bs + j)
                slot_tgt.append(6 * bs + 2 + s)
                if o == O_MAX:
                    a *= BN_SCALE
                alfc_t[ns["cheap"]] = a
                ns["cheap"] += 1
            # gather idx: wave of 2 slots, chunks of 128 elems (8/state)
            for wv in range((len(slot_src) + 1) // 2):
                j1 = slot_src[2 * wv]
                j2 = slot_src[2 * wv + 1] if 2 * wv + 1 < len(slot_src) else 0
                vals = np.concatenate([8 * j1 + np.arange(8),
                                       8 * j2 + np.arange(8)])
                idx_t[:, ns["wave"]] = vals[np.arange(128) % 16]
                ns["wave"] += 1
            # scatter idx: 16 chunks of 64 elems at target state
            for t in slot_tgt:
                vals = 16 * t + np.arange(16)
                six_t[:, ns["slot"]] = vals[np.arange(128) % 16]
                ns["slot"] += 1

        import ml_dtypes
        in_maps.append({
            "x0": s0[samples].reshape(BL, 4, 128, HW).astype(np.float32),
            "x1": s1[samples].reshape(BL, 4, 128, HW).astype(np.float32),
            "prew": prew,
            "fw": fw.astype(ml_dtypes.bfloat16),
            "pw": pw.astype(ml_dtypes.bfloat16),
            "dwt": dwt,
            "alf": np.broadcast_to(alf_t, (128, n["conv"])).copy(),
            "alfc": np.broadcast_to(alfc_t, (128, n["cheap"])).copy(),
            "idx": idx_t, "six": six_t, "rmap": rmap,
        })
    return in_maps, plan


_prog_cache = {}


def _get_program(plan):
    key = plan["key"]
    if key not in _prog_cache:
        _prog_cache[key] = build_program(plan)
    return _prog_cache[key]


def prepare_run(inputs):
    in_maps, plan = host_prepare(inputs)
    return in_maps, _get_program(plan)


def kernel(**inputs):
    in_maps, plan = host_prepare(inputs)
    nc = _get_program(plan)
    res = run_bass_kernel_spmd(nc, in_maps, core_ids=list(range(N_CORES)))
    out = np.empty((B, 512, H, W), np.float32)
    for core in range(N_CORES):
        o = res.results[core]["out"]
        for bs in range(BL):
            out[plan["assign"][core][bs]] = (
                o[bs].reshape(512, H, W).astype(np.float32))
    return out


# revision 27
# speedup vs baseline: 1.4727x; 1.4727x over previous
"""DARTS-cell (moe_routing) Trainium2 kernel — sparse per-core slot grid.

Data-parallel over batch B=32 across 8 cores (4 samples/core). Top-2-of-8
routing: only ~1.77 of 7 branches are active per (m, sample). The program
is specialized at runtime to a padded per-(step, branch-type) slot grid
sized by the max count over cores (SPMD: one program, per-core DATA
selects the work). Slots are shared across the core's 4 samples:

- slot inputs: gpsimd.ap_gather from the state stack [128, 4*6, 1024]
  (bf16), indices host-packed per core,
- slot weights/alphas: host-gathered per-slot DRAM tables,
- slot outputs: psum -> bf16 outbuf -> gpsimd.scatter_add into the
  data-dependent target state (16 distinct chunks per scatter, so no
  duplicate-index ambiguity),
- dummy (padding) slots run with zero weights/alphas and add zeros.

Conv branch forms are chosen per type at plan time (PE tap-fused matmuls
vs bf16 DVE dw-chains + pw matmul) to balance engine load. All branches
share one zero-margin z-buffer geometry [128,40,40], interior [4,36).
BN (eval, affine=False) folded into weights/maps on host.
"""

import sys

sys.path.insert(0, "/opt/trn_rl_repo")

import numpy as np
from concourse import bacc, mybir, tile
from concourse.bass_utils import run_bass_kernel_spmd

STEPS = 4
N_MIX = 14
OFFSETS = [0, 2, 5, 9]
B, C_IN, C, H, W = 32, 512, 128, 32, 32
HW = H * W
N_CORES = 8
BL = B // N_CORES
BN_SCALE = float(1.0 / np.sqrt(1.0 + 1e-5))

F32 = mybir.dt.float32
F32R = mybir.dt.float32r
BF16 = mybir.dt.bfloat16
I16 = mybir.dt.int16
ALU = mybir.AluOpType
ACTF = mybir.ActivationFunctionType

O_MAX, O_AVG, O_SKIP, O_SEP3, O_SEP5, O_DIL3, O_DIL5 = 1, 2, 3, 4, 5, 6, 7
CONV_OPS = (O_SEP5, O_SEP3, O_DIL5, O_DIL3)
CHEAP_OPS = (O_MAX, O_AVG, O_SKIP)
TYPE_NAME = {O_SEP3: "s3", O_SEP5: "s5", O_DIL3: "d3", O_DIL5: "d5",
             O_MAX: "max", O_AVG: "avg", O_SKIP: "skip"}
CONV_GEO = {O_SEP3: (3, 1, 1), O_SEP5: (5, 2, 1),
            O_DIL3: (3, 2, 2), O_DIL5: (5, 4, 2)}
TWO_STAGE = {O_SEP3, O_SEP5}

# unit costs (us) for form selection / balancing (measured-calibrated)
UC_PE = {O_SEP5: 27.8, O_DIL5: 13.9, O_SEP3: 10.0, O_DIL3: 5.0}
UC_DVE = {O_SEP5: 27.7, O_DIL5: 13.9, O_SEP3: 10.0, O_DIL3: 5.0}  # bf16 2x
UC_PW = {O_SEP5: 2.2, O_DIL5: 1.1, O_SEP3: 2.2, O_DIL3: 1.1}
UC_RELU = {O_SEP5: 1.7, O_DIL5: 0.85, O_SEP3: 1.7, O_DIL3: 0.85}


def _host_alphas(gates, top):
    g = np.asarray(gates).astype(np.float64)
    idx = np.argsort(-g, axis=-1, kind="stable")[..., :top]
    mask = np.zeros(g.shape, bool)
    np.put_along_axis(mask, idx, True, axis=-1)
    gm = np.where(mask, g, -np.inf)
    gm -= gm.max(axis=-1, keepdims=True)
    e = np.exp(gm)
    p = e / e.sum(axis=-1, keepdims=True)
    return p.astype(np.float32)


# ---------------------------------------------------------------- planning

def _grid(act, assign):
    """K[(s,o)] = max over cores of per-step per-type item count."""
    K = {}
    for s in range(STEPS):
        for o in CONV_OPS + CHEAP_OPS:
            mx = 0
            for core in range(N_CORES):
                c = sum(int(act[OFFSETS[s] + j, smp, o])
                        for smp in assign[core] for j in range(2 + s))
                if c > mx:
                    mx = c
            K[(s, o)] = mx
    return K


def _grid_cost(act, assign):
    K = _grid(act, assign)
    pe = dve = 0.0
    for (s, o), k in K.items():
        if o in UC_PE:
            pe += k * UC_PE[o]
        else:
            dve += k * 3.0
    return max(pe, dve) + 0.15 * (pe + dve), K


def _optimize_assignment(act, iters=4000, seed=0):
    rng = np.random.default_rng(seed)
    w = np.zeros(B)
    for m in range(N_MIX):
        for b in range(B):
            w[b] += sum(UC_PE[o] for o in CONV_OPS if act[m, b, o])
    order = np.argsort(-w)
    loads = [0.0] * N_CORES
    assign = [[] for _ in range(N_CORES)]
    for b in order:
        c = min((i for i in range(N_CORES) if len(assign[i]) < BL),
                key=lambda i: loads[i])
        assign[c].append(int(b))
        loads[c] += w[b]
    cur = [list(a) for a in assign]
    cur_cost, _ = _grid_cost(act, cur)
    best, best_cost = [list(a) for a in cur], cur_cost
    for it in range(iters):
        c1, c2 = rng.integers(0, N_CORES, 2)
        s1, s2 = rng.integers(0, BL, 2)
        if c1 == c2:
            continue
        cur[c1][s1], cur[c2][s2] = cur[c2][s2], cur[c1][s1]
        cost, _ = _grid_cost(act, cur)
        if cost <= cur_cost:
            cur_cost = cost
            if cost < best_cost:
                best_cost, best = cost, [list(a) for a in cur]
        else:
            cur[c1][s1], cur[c2][s2] = cur[c2][s2], cur[c1][s1]
    return best, _grid(act, best)


def _pick_forms(K):
    """Choose pe/dve form per conv type + avg engine to balance loads."""
    n = {o: sum(K[(s, o)] for s in range(STEPS)) for o in CONV_OPS + CHEAP_OPS}
    n_conv = sum(n[o] for o in CONV_OPS)
    n_cheap = sum(n[o] for o in CHEAP_OPS)
    n_slots = n_conv + n_cheap
    best = None
    import itertools
    for combo in itertools.product(("pe", "dve"), repeat=4):
        forms = dict(zip(CONV_OPS, combo))
        for avg_eng in ("gpsimd", "dve"):
            pe = 34.0 + sum(n[o] * UC_PE[o] for o in CONV_OPS
                            if forms[o] == "pe")
            pe += sum(n[o] * UC_PW[o] for o in CONV_OPS if forms[o] == "dve")
            dve = sum(n[o] * UC_DVE[o] for o in CONV_OPS if forms[o] == "dve")
            dve += n[O_MAX] * 2.8
            dve += n[O_AVG] * (3.4 if avg_eng == "dve" else 0.6)
            gps = n_slots / 2 * 2.85 + n_slots * 1.42 + 30.0
            gps += n[O_AVG] * (10.8 if avg_eng == "gpsimd" else 0.0)
            sca = 60.0 + sum(n[o] * UC_RELU[o] for o in CONV_OPS)
            sca += n_conv * 0.85 + n_cheap * 0.85
            span = max(pe, dve, gps, sca) + 0.10 * (pe + dve + gps + sca)
            if best is None or span < best[0]:
                best = (span, forms, avg_eng, dict(pe=pe, dve=dve, gps=gps,
                                                   sca=sca))
    return best[1], best[2], best[3]


def build_plan(gates, top):
    p = _host_alphas(gates, top)
    act = p > 0
    assign, K = _optimize_assignment(act)
    forms, avg_eng, loads = _pick_forms(K)

    # per-step emission schedule: conv types round-robin, then cheap
    sched = {}
    for s in range(STEPS):
        convs = []
        rem = {o: K[(s, o)] for o in CONV_OPS}
        while any(rem.values()):
            for o in CONV_OPS:
                if rem[o]:
                    convs.append(o)
                    rem[o] -= 1
        cheaps = []
        for o in CHEAP_OPS:
            cheaps += [o] * K[(s, o)]
        sched[s] = (tuple(convs), tuple(cheaps))

    n_stage = n_pw = n_dve = n_conv = n_cheap = n_wave = 0
    for s in range(STEPS):
        convs, cheaps = sched[s]
        for o in convs:
            if forms[o] == "pe":
                n_stage += 2 if o in TWO_STAGE else 1
            else:
                n_pw += 2 if o in TWO_STAGE else 1
                n_dve += 1
        n_conv += len(convs)
        n_cheap += len(cheaps)
        n_wave += (len(convs) + len(cheaps) + 1) // 2

    key = (tuple(sorted(K.items())), tuple(sorted(forms.items())), avg_eng)
    return dict(p=p, act=act, assign=assign, K=K, sched=sched, forms=forms,
                avg_eng=avg_eng, loads=loads, key=key,
                n=dict(stage=max(n_stage, 1), pw=max(n_pw, 1),
                       dve=max(n_dve, 1), conv=max(n_conv, 1),
                       cheap=max(n_cheap, 1), wave=max(n_wave, 1),
                       slots=max(n_conv + n_cheap, 1)))


# ---------------------------------------------------------------- program

def build_program(plan, n_cores=N_CORES):
    sched, forms, avg_eng, n = (plan["sched"], plan["forms"],
                                plan["avg_eng"], plan["n"])
    nc = bacc.Bacc("TRN2", target_bir_lowering=False, debug=False,
                   num_devices=n_cores)

    x0_d = nc.dram_tensor("x0", [BL, 4, 128, HW], F32, kind="ExternalInput").ap()
    x1_d = nc.dram_tensor("x1", [BL, 4, 128, HW], F32, kind="ExternalInput").ap()
    prew_d = nc.dram_tensor("prew", [128, 2, 4, 128], F32R, kind="ExternalInput").ap()
    fw_d = nc.dram_tensor("fw", [128, n["stage"], 25, 128], BF16, kind="ExternalInput").ap()
    pw_d = nc.dram_tensor("pw", [128, n["pw"], 128], BF16, kind="ExternalInput").ap()
    dwt_d = nc.dram_tensor("dwt", [128, n["dve"], 50], F32, kind="ExternalInput").ap()
    alf_d = nc.dram_tensor("alf", [128, n["conv"]], F32, kind="ExternalInput").ap()
    alfc_d = nc.dram_tensor("alfc", [128, n["cheap"]], F32, kind="ExternalInput").ap()
    idx_d = nc.dram_tensor("idx", [128, n["wave"]], I16, kind="ExternalInput").ap()
    six_d = nc.dram_tensor("six", [128, n["slots"]], I16, kind="ExternalInput").ap()
    rmap_d = nc.dram_tensor("rmap", [128, 32, 32], F32, kind="ExternalInput").ap()
    out_d = nc.dram_tensor("out", [BL, 4, 128, HW], F32, kind="ExternalOutput").ap()

    with tile.TileContext(nc) as tc:
        with (
            tc.tile_pool(name="const", bufs=1) as cpool,
            tc.tile_pool(name="work", bufs=1) as wpool,
            tc.tile_pool(name="xs", bufs=2) as xpool,
            tc.tile_pool(name="stg", bufs=3) as spool,
            tc.tile_pool(name="dwa", bufs=3) as dpool,
            tc.tile_pool(name="pwb", bufs=2) as pwpool,
            tc.tile_pool(name="fw", bufs=2) as fwpool,
            tc.tile_pool(name="ob", bufs=8) as obpool,
            tc.tile_pool(name="ost", bufs=2) as opool,
            tc.tile_pool(name="ps_state", bufs=2, space="PSUM") as pspool,
            tc.tile_pool(name="ps_scr", bufs=2, space="PSUM") as scrpool,
        ):
            prew = cpool.tile([128, 2, 4, 128], F32R, tag="prew")
            dwt = cpool.tile([128, n["dve"], 50], F32, tag="dwt")
            alf = cpool.tile([128, n["conv"]], F32, tag="alf")
            alfc = cpool.tile([128, n["cheap"]], F32, tag="alfc")
            idx = cpool.tile([128, n["wave"]], I16, tag="idx")
            six = cpool.tile([128, n["slots"]], I16, tag="six")
            rmap = cpool.tile([128, 32, 32], F32, tag="rmap")
            for t, d in ((prew, prew_d), (dwt, dwt_d), (alf, alf_d),
                         (alfc, alfc_d), (idx, idx_d), (six, six_d),
                         (rmap, rmap_d)):
                nc.sync.dma_start(t[:], d)

            # state stack: 4 samples x 6 states, bf16
            states = wpool.tile([128, 24, 1024], BF16, tag="states")
            # per-step scatter target (separate tile so slot-output scatters
            # never alias the gathers reading `states`)
            newstate = wpool.tile([128, 4, 1024], BF16, tag="newstate")

            # pool scratch
            xpmax = wpool.tile([128, 34, 34], BF16, tag="xpmax")
            rmpad = wpool.tile([128, 34, 32], BF16, tag="rmpad")
            ptmp0 = wpool.tile([128, 32, 32], BF16, tag="ptmp0")
            xpsum = wpool.tile([128, 34, 34], F32, tag="xpsum")
            rspad = wpool.tile([128, 34, 32], F32, tag="rspad")
            ptmp1 = wpool.tile([128, 32, 32], F32, tag="ptmp1")
            nc.gpsimd.memset(xpmax[:], -1e30)
            nc.gpsimd.memset(rmpad[:], -1e30)
            nc.gpsimd.memset(xpsum[:], 0.0)
            nc.gpsimd.memset(rspad[:], 0.0)

            zbufs = [wpool.tile([128, 40, 40], BF16, tag=f"z{i}",
                                name=f"z{i}") for i in range(4)]
            for z in zbufs:
                nc.gpsimd.memset(z[:], 0.0)
            zctr = [0]

            def flat(ap3):
                return ap3.rearrange("p a b -> p (a b)")

            def relu_into_z(src_ap, scale):
                z = zbufs[zctr[0] % len(zbufs)]
                zctr[0] += 1
                nc.scalar.activation(z[:, 4:36, 4:36], src_ap, ACTF.Relu,
                                     scale=scale)
                return z

            def mm_chunks(psum3, lhsT, rhs3, flags):
                s0, e0, s1, e1 = flags
                nc.tensor.matmul(psum3[:, 0:16, :], lhsT, rhs3[:, 0:16, :],
                                 start=s0, stop=e0)
                nc.tensor.matmul(psum3[:, 16:32, :], lhsT, rhs3[:, 16:32, :],
                                 start=s1, stop=e1)

            def dw_chain(z, dslot, tap0, k, pad, stride):
                dwacc = dpool.tile([128, 32, 32], BF16, tag="dwacc")
                first = True
                for ky in range(k):
                    for kx in range(k):
                        t = tap0 + ky * k + kx
                        y0 = 4 - pad + stride * ky
                        x0 = 4 - pad + stride * kx
                        view = z[:, y0:y0 + 32, x0:x0 + 32]
                        sc = dwt[:, dslot, t:t + 1]
                        if first:
                            nc.vector.tensor_scalar_mul(dwacc[:], view, sc)
                            first = False
                        else:
                            nc.vector.scalar_tensor_tensor(
                                dwacc[:], view, sc, dwacc[:],
                                op0=ALU.mult, op1=ALU.add)
                return dwacc

            def fused_stage(stage_i, z, k, pad, stride, psum3):
                taps = k * k
                half = (taps + 1) // 2
                for (a, e) in ((0, half), (half, taps)):
                    fwt = fwpool.tile([128, 13, 128], BF16, tag="fw")
                    nc.sync.dma_start(fwt[:, 0:e - a, :],
                                      fw_d[:, stage_i, a:e, :])
                    for t in range(a, e):
                        ky, kx = divmod(t, k)
                        y0 = 4 - pad + stride * ky
                        x0 = 4 - pad + stride * kx
                        for h2 in range(2):
                            nc.tensor.matmul(
                                psum3[:, 16 * h2:16 * h2 + 16, :],
                                fwt[:, t - a, :],
                                z[:, y0 + 16 * h2:y0 + 16 * h2 + 16,
                                  x0:x0 + 32],
                                start=(t == 0), stop=(t == taps - 1))
                return psum3

            def stream_pw(pw_i):
                t = pwpool.tile([128, 1, 128], BF16, tag="pwb")
                nc.sync.dma_start(t[:], pw_d[:, pw_i:pw_i + 1, :])
                return t[:, 0, :]

            def conv_slot(o, x_ap, cs, ctr):
                """Emit conv slot; returns stp psum [128,32,32] result."""
                k, pad, stride = CONV_GEO[o]
                a_ap = alf[:, cs:cs + 1]
                stp = pspool.tile([128, 32, 32], F32, tag="stp")
                if forms[o] == "pe":
                    z1 = relu_into_z(x_ap, a_ap)
                    if o in TWO_STAGE:
                        scr = scrpool.tile([128, 32, 32], F32, tag="scr")
                        fused_stage(ctr["stage"], z1, k, pad, stride, scr)
                        ctr["stage"] += 1
                        z2 = relu_into_z(scr[:], 1.0)
                        fused_stage(ctr["stage"], z2, k, pad, stride, stp)
                        ctr["stage"] += 1
                    else:
                        fused_stage(ctr["stage"], z1, k, pad, stride, stp)
                        ctr["stage"] += 1
                else:
                    z1 = relu_into_z(x_ap, a_ap)
                    dwacc = dw_chain(z1, ctr["dve"], 0, k, pad, stride)
                    if o in TWO_STAGE:
                        scr = scrpool.tile([128, 32, 32], F32, tag="scr")
                        mm_chunks(scr, stream_pw(ctr["pw"]), dwacc,
                                  (True, True, True, True))
                        ctr["pw"] += 1
                        z2 = relu_into_z(scr[:], 1.0)
                        dwacc2 = dw_chain(z2, ctr["dve"], 25, k, pad, stride)
                        mm_chunks(stp, stream_pw(ctr["pw"]), dwacc2,
                                  (True, True, True, True))
                        ctr["pw"] += 1
                    else:
                        mm_chunks(stp, stream_pw(ctr["pw"]), dwacc,
                                  (True, True, True, True))
                        ctr["pw"] += 1
                    ctr["dve"] += 1
                return stp

            def cheap_slot(o, x_ap, cc_i, ob):
                """Compute cheap op into ob tile [128,32,32] (alpha-scaled)."""
                sc = alfc[:, cc_i:cc_i + 1]
                dst = ob[:]
                if o == O_SKIP:
                    nc.scalar.activation(dst, x_ap, ACTF.Copy, scale=sc)
                elif o == O_MAX:
                    nc.scalar.copy(xpmax[:, 1:33, 1:33], x_ap)
                    t = ptmp0
                    nc.vector.tensor_max(t[:], xpmax[:, 1:33, 0:32],
                                         xpmax[:, 1:33, 1:33])
                    nc.vector.tensor_max(rmpad[:, 1:33, :], t[:],
                                         xpmax[:, 1:33, 2:34])
                    nc.vector.tensor_max(t[:], rmpad[:, 0:32, :],
                                         rmpad[:, 1:33, :])
                    nc.vector.tensor_max(t[:], t[:], rmpad[:, 2:34, :])
                    nc.scalar.activation(dst, t[:], ACTF.Copy, scale=sc)
                else:  # O_AVG
                    eng = nc.gpsimd if avg_eng == "gpsimd" else nc.vector
                    nc.scalar.copy(xpsum[:, 1:33, 1:33], x_ap)
                    t = ptmp1
                    eng.tensor_add(t[:], xpsum[:, 1:33, 0:32],
                                   xpsum[:, 1:33, 1:33])
                    eng.tensor_add(rspad[:, 1:33, :], t[:],
                                   xpsum[:, 1:33, 2:34])
                    eng.tensor_add(t[:], rspad[:, 0:32, :],
                                   rspad[:, 1:33, :])
                    eng.tensor_add(t[:], t[:], rspad[:, 2:34, :])
                    eng.tensor_mul(t[:], t[:], rmap[:])
                    nc.scalar.activation(dst, t[:], ACTF.Copy, scale=sc)

            # ---- preprocess ----
            for bs in range(BL):
                for inp, xd in ((0, x0_d), (1, x1_d)):
                    scr = scrpool.tile([128, 32, 32], F32, tag="scr")
                    for kc in range(4):
                        xb = xpool.tile([128, HW], F32, tag="xb")
                        nc.sync.dma_start(xb[:], xd[bs, kc])
                        xr = xpool.tile([128, HW], F32R, tag="xr")
                        nc.scalar.activation(xr[:], xb[:], ACTF.Relu)
                        for h in range(2):
                            nc.tensor.matmul(
                                scr[:, 16 * h:16 * (h + 1), :],
                                prew[:, inp, kc, :],
                                xr[:, 512 * h:512 * (h + 1)].rearrange(
                                    "p (a c) -> p a c", a=16),
                                start=(kc == 0), stop=(kc == 3))
                    nc.scalar.copy(states[:, 6 * bs + inp, :].rearrange(
                        "p (h w) -> p h w", h=32), scr[:])

            # ---- steps ----
            ctr = dict(stage=0, pw=0, dve=0)
            n_slot_c = [0]
            n_conv_c = n_cheap_c = n_wave_c = 0
            for s in range(STEPS):
                convs, cheaps = sched[s]
                n_slots = len(convs) + len(cheaps)
                n_waves = (n_slots + 1) // 2
                stgs = {}

                def slot_x(i):
                    wv = i // 2
                    if wv not in stgs:
                        stg = spool.tile([128, 16, 128], BF16, tag="stg")
                        nc.gpsimd.ap_gather(
                            flat(stg[:]), flat(states[:]),
                            idx[:, n_wave_c + wv:n_wave_c + wv + 1],
                            channels=128, num_elems=192, d=128, num_idxs=16)
                        stgs[wv] = stg
                    stg = stgs[wv]
                    half = stg[:, 8 * (i % 2):8 * (i % 2) + 8, :]
                    return flat(half).rearrange("p (h w) -> p h w", h=32)

                nc.vector.memset(newstate[:], 0.0)

                def scatter(ob, si):
                    nc.gpsimd.scatter_add(
                        flat(newstate[:]).rearrange("p (a b) -> p a b", b=64),
                        six[:, si:si + 1],
                        flat(ob[:]).rearrange("p (a b) -> p a b", b=64),
                        channels=128, num_elems=64, d=64, num_idxs=16)

                # scatter of slot i is emitted after slot i+1's compute so
                # the (in-order) gpsimd queue isn't head-of-line blocked on
                # slot i's psum evac while later gathers wait behind it.
                pend = []

                def flush(keep):
                    while len(pend) > keep:
                        scatter(*pend.pop(0))

                for i, o in enumerate(convs):
                    stp = conv_slot(o, slot_x(i), n_conv_c + i, ctr)
                    ob = obpool.tile([128, 32, 32], BF16, tag="ob")
                    nc.scalar.copy(ob[:], stp[:])
                    pend.append((ob, n_slot_c[0] + i))
                    flush(1)
                for i, o in enumerate(cheaps):
                    ob = obpool.tile([128, 32, 32], BF16, tag="ob")
                    cheap_slot(o, slot_x(len(convs) + i), n_cheap_c + i, ob)
                    pend.append((ob, n_slot_c[0] + len(convs) + i))
                    flush(1)
                flush(0)
                n_conv_c += len(convs)
                n_cheap_c += len(cheaps)
                n_wave_c += n_waves
                n_slot_c[0] += n_slots

                for bs in range(BL):
                    nc.scalar.copy(states[:, 6 * bs + 2 + s, :],
                                   newstate[:, bs, :])
                    ost = opool.tile([128, 1024], F32, tag="ost")
                    nc.scalar.copy(ost[:], newstate[:, bs, :])
                    nc.sync.dma_start(out_d[bs, s], ost[:])

    nc.compile()
    return nc


# ---------------------------------------------------------------- host data

def host_prepare(inputs):
    s0, s1 = np.asarray(inputs["s0"]), np.asarray(inputs["s1"])
    gates = np.asarray(inputs["gates"])
    top = int(inputs["top"])
    plan = build_plan(gates, top)
    p, assign, sched, forms, n = (plan["p"], plan["assign"], plan["sched"],
                                  plan["forms"], plan["n"])

    prew = np.empty((128, 2, 4, 128), np.float32)
    for inp, wname in ((0, "pre0_w"), (1, "pre1_w")):
        wmat = np.asarray(inputs[wname]) * BN_SCALE
        for kc in range(4):
            prew[:, inp, kc, :] = wmat[:, 128 * kc:128 * (kc + 1)].T

    FUSE_KEYS = {O_SEP5: (("sep5_pw1", "sep5_dw1"), ("sep5_pw2", "sep5_dw2")),
                 O_DIL5: (("dil5_pw", "dil5_dw"),),
                 O_SEP3: (("sep3_pw1", "sep3_dw1"), ("sep3_pw2", "sep3_dw2")),
                 O_DIL3: (("dil3_pw", "dil3_dw"),)}

    def fuse(pw_key, dw_key, m, k):
        pwm = np.asarray(inputs[pw_key])[m].astype(np.float32) * BN_SCALE
        dwm = np.asarray(inputs[dw_key])[m].astype(np.float32).reshape(C, k * k)
        return pwm.T[:, None, :] * dwm[:, :, None]  # [ci, k*k, co]

    cnt = np.zeros((32, 32), np.float32)
    for dy in (-1, 0, 1):
        for dx in (-1, 0, 1):
            cnt[max(0, dy):32 - max(0, -dy),
                max(0, dx):32 - max(0, -dx)] += 1
    rmap = np.broadcast_to((BN_SCALE / cnt).astype(np.float32),
                           (128, 32, 32)).copy()

    act = plan["act"]
    in_maps = []
    for core in range(N_CORES):
        samples = assign[core]
        fw = np.zeros((128, n["stage"], 25, 128), np.float32)
        pw = np.zeros((128, n["pw"], 128), np.float32)
        dwt = np.zeros((128, n["dve"], 50), np.float32)
        alf_t = np.zeros((n["conv"],), np.float32)
        alfc_t = np.zeros((n["cheap"],), np.float32)
        idx_t = np.zeros((128, n["wave"]), np.int16)
        six_t = np.zeros((128, n["slots"]), np.int16)
        ns = dict(stage=0, pw=0, dve=0, conv=0, cheap=0, wave=0, slot=0)

        for s in range(STEPS):
            convs, cheaps = sched[s]
            items = {o: [] for o in CONV_OPS + CHEAP_OPS}
            for bs in range(BL):
                smp = samples[bs]
                for j in range(2 + s):
                    m = OFFSETS[s] + j
                    for o in CONV_OPS + CHEAP_OPS:
                        if act[m, smp, o]:
                            items[o].append((m, j, bs))
            used = {o: 0 for o in items}
            slot_src = []   # gather chunk base per slot (j-state of its bs)
            slot_tgt = []   # scatter target state index per slot
            for o in convs:
                if used[o] < len(items[o]):
                    m, j, bs = items[o][used[o]]
                    used[o] += 1
                    a = float(p[m, samples[bs], o])
                else:
                    m, j, bs, a = None, 0, 0, 0.0
                slot_src.append(6 * bs + j)
                slot_tgt.append(bs)
                alf_t[ns["conv"]] = a
                k, _, _ = CONV_GEO[o]
                if forms[o] == "pe":
                    for st_i, (pwk, dwk) in enumerate(FUSE_KEYS[o]):
                        if m is not None:
                            fw[:, ns["stage"], 0:k * k] = fuse(pwk, dwk, m, k)
                        ns["stage"] += 1
                else:
                    if m is not None:
                        for st_i, (pwk, dwk) in enumerate(FUSE_KEYS[o]):
                            dwm = np.asarray(inputs[dwk])[m].reshape(C, k * k)
                            dwt[:, ns["dve"], 25 * st_i:25 * st_i + k * k] = dwm
                            pw[:, ns["pw"] + st_i] = (
                                np.asarray(inputs[pwk])[m].T * BN_SCALE)
                    ns["pw"] += 2 if o in TWO_STAGE else 1
                    ns["dve"] += 1
                ns["conv"] += 1
            for o in cheaps:
                if used[o] < len(items[o]):
                    m, j, bs = items[o][used[o]]
                    used[o] += 1
                    a = float(p[m, samples[bs], o])
                else:
                    m, j, bs, a = None, 0, 0, 0.0
                slot_src.append(6 * bs + j)
                slot_tgt.append(bs)
                if o == O_MAX:
                    a *= BN_SCALE
                alfc_t[ns["cheap"]] = a
                ns["cheap"] += 1
            # gather idx: wave of 2 slots, chunks of 128 elems (8/state)
            for wv in range((len(slot_src) + 1) // 2):
                j1 = slot_src[2 * wv]
                j2 = slot_src[2 * wv + 1] if 2 * wv + 1 < len(slot_src) else 0
                vals = np.concatenate([8 * j1 + np.arange(8),
                                       8 * j2 + np.arange(8)])
                idx_t[:, ns["wave"]] = vals[np.arange(128) % 16]
                ns["wave"] += 1
            # scatter idx: 16 chunks of 64 elems at target state
            for t in slot_tgt:
                vals = 16 * t + np.arange(16)
                six_t[:, ns["slot"]] = vals[np.arange(128) % 16]
                ns["slot"] += 1

        import ml_dtypes
        in_maps.append({
            "x0": s0[samples].reshape(BL, 4, 128, HW).astype(np.float32),
            "x1": s1[samples].reshape(BL, 4, 128, HW).astype(np.float32),
            "prew": prew,
            "fw": fw.astype(ml_dtypes.bfloat16),
            "pw": pw.astype(ml_dtypes.bfloat16),
            "dwt": dwt,
            "alf": np.broadcast_to(alf_t, (128, n["conv"])).copy(),
            "alfc": np.broadcast_to(alfc_t, (128, n["cheap"])).copy(),
            "idx": idx_t, "six": six_t, "rmap": rmap,
        })
    return in_maps, plan


_prog_cache = {}


def _get_program(plan):
    key = plan["key"]
    if key not in _prog_cache:
        _prog_cache[key] = build_program(plan)
    return _prog_cache[key]


def prepare_run(inputs):
    in_maps, plan = host_prepare(inputs)
    return in_maps, _get_program(plan)


def kernel(**inputs):
    in_maps, plan = host_prepare(inputs)
    nc = _get_program(plan)
    res = run_bass_kernel_spmd(nc, in_maps, core_ids=list(range(N_CORES)))
    out = np.empty((B, 512, H, W), np.float32)
    for core in range(N_CORES):
        o = res.results[core]["out"]
        for bs in range(BL):
            out[plan["assign"][core][bs]] = (
                o[bs].reshape(512, H, W).astype(np.float32))
    return out


# revision 36
# speedup vs baseline: 1.5132x; 1.0275x over previous
"""DARTS-cell (moe_routing) Trainium2 kernel — sparse per-core slot grid.

Data-parallel over batch B=32 across 8 cores (4 samples/core). Top-2-of-8
routing: only ~1.77 of 7 branches are active per (m, sample). The program
is specialized at runtime to a padded per-(step, branch-type) slot grid
sized by the max count over cores (SPMD: one program, per-core DATA
selects the work). Slots are shared across the core's 4 samples:

- slot inputs: gpsimd.ap_gather from the state stack [128, 4*6, 1024]
  (bf16), indices host-packed per core,
- slot weights/alphas: host-gathered per-slot DRAM tables,
- slot outputs: psum -> bf16 outbuf -> gpsimd.scatter_add into the
  data-dependent target state (16 distinct chunks per scatter, so no
  duplicate-index ambiguity),
- dummy (padding) slots run with zero weights/alphas and add zeros.

Conv branch forms are chosen per type at plan time (PE tap-fused matmuls
vs bf16 DVE dw-chains + pw matmul) to balance engine load. All branches
share one zero-margin z-buffer geometry [128,40,40], interior [4,36).
BN (eval, affine=False) folded into weights/maps on host.
"""

import sys

sys.path.insert(0, "/opt/trn_rl_repo")

import numpy as np
from concourse import bacc, mybir, tile
from concourse.bass_utils import run_bass_kernel_spmd

STEPS = 4
N_MIX = 14
OFFSETS = [0, 2, 5, 9]
B, C_IN, C, H, W = 32, 512, 128, 32, 32
HW = H * W
N_CORES = 8
BL = B // N_CORES
BN_SCALE = float(1.0 / np.sqrt(1.0 + 1e-5))

F32 = mybir.dt.float32
F32R = mybir.dt.float32r
BF16 = mybir.dt.bfloat16
I16 = mybir.dt.int16
ALU = mybir.AluOpType
ACTF = mybir.ActivationFunctionType

O_MAX, O_AVG, O_SKIP, O_SEP3, O_SEP5, O_DIL3, O_DIL5 = 1, 2, 3, 4, 5, 6, 7
CONV_OPS = (O_SEP5, O_SEP3, O_DIL5, O_DIL3)
CHEAP_OPS = (O_MAX, O_AVG, O_SKIP)
TYPE_NAME = {O_SEP3: "s3", O_SEP5: "s5", O_DIL3: "d3", O_DIL5: "d5",
             O_MAX: "max", O_AVG: "avg", O_SKIP: "skip"}
CONV_GEO = {O_SEP3: (3, 1, 1), O_SEP5: (5, 2, 1),
            O_DIL3: (3, 2, 2), O_DIL5: (5, 4, 2)}
TWO_STAGE = {O_SEP3, O_SEP5}

# unit costs (us) for form selection / balancing (measured-calibrated)
UC_PE = {O_SEP5: 27.8, O_DIL5: 13.9, O_SEP3: 10.0, O_DIL3: 5.0}
UC_DVE = {O_SEP5: 27.7, O_DIL5: 13.9, O_SEP3: 10.0, O_DIL3: 5.0}  # bf16 2x
UC_PW = {O_SEP5: 2.2, O_DIL5: 1.1, O_SEP3: 2.2, O_DIL3: 1.1}
UC_RELU = {O_SEP5: 1.7, O_DIL5: 0.85, O_SEP3: 1.7, O_DIL3: 0.85}


def _host_alphas(gates, top):
    g = np.asarray(gates).astype(np.float64)
    idx = np.argsort(-g, axis=-1, kind="stable")[..., :top]
    mask = np.zeros(g.shape, bool)
    np.put_along_axis(mask, idx, True, axis=-1)
    gm = np.where(mask, g, -np.inf)
    gm -= gm.max(axis=-1, keepdims=True)
    e = np.exp(gm)
    p = e / e.sum(axis=-1, keepdims=True)
    return p.astype(np.float32)


# ---------------------------------------------------------------- planning

def _grid(act, assign):
    """K[(s,o)] = max over cores of per-step per-type item count."""
    K = {}
    for s in range(STEPS):
        for o in CONV_OPS + CHEAP_OPS:
            mx = 0
            for core in range(N_CORES):
                c = sum(int(act[OFFSETS[s] + j, smp, o])
                        for smp in assign[core] for j in range(2 + s))
                if c > mx:
                    mx = c
            K[(s, o)] = mx
    return K


def _grid_cost(act, assign):
    K = _grid(act, assign)
    pe = dve = 0.0
    for (s, o), k in K.items():
        if o in UC_PE:
            pe += k * UC_PE[o]
        else:
            dve += k * 3.0
    return max(pe, dve) + 0.15 * (pe + dve), K


def _optimize_assignment(act, iters=4000, seed=0):
    rng = np.random.default_rng(seed)
    w = np.zeros(B)
    for m in range(N_MIX):
        for b in range(B):
            w[b] += sum(UC_PE[o] for o in CONV_OPS if act[m, b, o])
    order = np.argsort(-w)
    loads = [0.0] * N_CORES
    assign = [[] for _ in range(N_CORES)]
    for b in order:
        c = min((i for i in range(N_CORES) if len(assign[i]) < BL),
                key=lambda i: loads[i])
        assign[c].append(int(b))
        loads[c] += w[b]
    cur = [list(a) for a in assign]
    cur_cost, _ = _grid_cost(act, cur)
    best, best_cost = [list(a) for a in cur], cur_cost
    for it in range(iters):
        c1, c2 = rng.integers(0, N_CORES, 2)
        s1, s2 = rng.integers(0, BL, 2)
        if c1 == c2:
            continue
        cur[c1][s1], cur[c2][s2] = cur[c2][s2], cur[c1][s1]
        cost, _ = _grid_cost(act, cur)
        if cost <= cur_cost:
            cur_cost = cost
            if cost < best_cost:
                best_cost, best = cost, [list(a) for a in cur]
        else:
            cur[c1][s1], cur[c2][s2] = cur[c2][s2], cur[c1][s1]
    return best, _grid(act, best)


def _pick_forms(K):
    """Choose pe/dve form per conv type + avg engine to balance loads."""
    n = {o: sum(K[(s, o)] for s in range(STEPS)) for o in CONV_OPS + CHEAP_OPS}
    n_conv = sum(n[o] for o in CONV_OPS)
    n_cheap = sum(n[o] for o in CHEAP_OPS)
    n_slots = n_conv + n_cheap
    best = None
    import itertools
    for combo in itertools.product(("pe", "dve"), repeat=4):
        forms = dict(zip(CONV_OPS, combo))
        for avg_eng in ("gpsimd", "dve"):
            pe = 34.0 + sum(n[o] * UC_PE[o] for o in CONV_OPS
                            if forms[o] == "pe")
            pe += sum(n[o] * UC_PW[o] for o in CONV_OPS if forms[o] == "dve")
            dve = sum(n[o] * UC_DVE[o] for o in CONV_OPS if forms[o] == "dve")
            dve += n[O_MAX] * 2.8
            dve += n[O_AVG] * (3.4 if avg_eng == "dve" else 0.6)
            gps = n_slots / 2 * 2.85 + n_slots * 1.42 + 30.0
            gps += n[O_AVG] * (10.8 if avg_eng == "gpsimd" else 0.0)
            sca = 60.0 + sum(n[o] * UC_RELU[o] for o in CONV_OPS)
            sca += n_conv * 0.85 + n_cheap * 0.85
            span = max(pe, dve, gps, sca) + 0.10 * (pe + dve + gps + sca)
            if best is None or span < best[0]:
                best = (span, forms, avg_eng, dict(pe=pe, dve=dve, gps=gps,
                                                   sca=sca))
    return best[1], best[2], best[3]


def build_plan(gates, top):
    p = _host_alphas(gates, top)
    act = p > 0
    assign, K = _optimize_assignment(act)
    forms, avg_eng, loads = _pick_forms(K)

    # per-step emission schedule: conv types round-robin, then cheap
    sched = {}
    for s in range(STEPS):
        convs = []
        rem = {o: K[(s, o)] for o in CONV_OPS}
        while any(rem.values()):
            for o in CONV_OPS:
                if rem[o]:
                    convs.append(o)
                    rem[o] -= 1
        cheaps = []
        for o in CHEAP_OPS:
            cheaps += [o] * K[(s, o)]
        sched[s] = (tuple(convs), tuple(cheaps))

    n_stage = n_pw = n_dve = n_conv = n_cheap = n_wave = 0
    for s in range(STEPS):
        convs, cheaps = sched[s]
        for o in convs:
            if forms[o] == "pe":
                n_stage += 2 if o in TWO_STAGE else 1
            else:
                n_pw += 2 if o in TWO_STAGE else 1
                n_dve += 1
        n_conv += len(convs)
        n_cheap += len(cheaps)
        n_wave += (len(convs) + len(cheaps) + 1) // 2

    key = (tuple(sorted(K.items())), tuple(sorted(forms.items())), avg_eng)
    return dict(p=p, act=act, assign=assign, K=K, sched=sched, forms=forms,
                avg_eng=avg_eng, loads=loads, key=key,
                n=dict(stage=max(n_stage, 1), pw=max(n_pw, 1),
                       dve=max(n_dve, 1), conv=max(n_conv, 1),
                       cheap=max(n_cheap, 1), wave=max(n_wave, 1),
                       slots=max(n_conv + n_cheap, 1)))


# ---------------------------------------------------------------- program

def build_program(plan, n_cores=N_CORES):
    sched, forms, avg_eng, n = (plan["sched"], plan["forms"],
                                plan["avg_eng"], plan["n"])
    nc = bacc.Bacc("TRN2", target_bir_lowering=False, debug=False,
                   num_devices=n_cores)

    x0_d = nc.dram_tensor("x0", [BL, 4, 128, HW], F32, kind="ExternalInput").ap()
    x1_d = nc.dram_tensor("x1", [BL, 4, 128, HW], F32, kind="ExternalInput").ap()
    prew_d = nc.dram_tensor("prew", [128, 2, 4, 128], F32R, kind="ExternalInput").ap()
    fw_d = nc.dram_tensor("fw", [128, n["stage"], 25, 128], BF16, kind="ExternalInput").ap()
    pw_d = nc.dram_tensor("pw", [128, n["pw"], 128], BF16, kind="ExternalInput").ap()
    dwt_d = nc.dram_tensor("dwt", [128, n["dve"], 50], F32, kind="ExternalInput").ap()
    alf_d = nc.dram_tensor("alf", [128, n["conv"]], F32, kind="ExternalInput").ap()
    alfc_d = nc.dram_tensor("alfc", [128, n["cheap"]], F32, kind="ExternalInput").ap()
    idx_d = nc.dram_tensor("idx", [128, n["wave"]], I16, kind="ExternalInput").ap()
    six_d = nc.dram_tensor("six", [128, n["slots"]], I16, kind="ExternalInput").ap()
    rmap_d = nc.dram_tensor("rmap", [128, 32, 32], F32, kind="ExternalInput").ap()
    out_d = nc.dram_tensor("out", [BL, 4, 128, HW], F32, kind="ExternalOutput").ap()

    with tile.TileContext(nc) as tc:
        with (
            tc.tile_pool(name="const", bufs=1) as cpool,
            tc.tile_pool(name="work", bufs=1) as wpool,
            tc.tile_pool(name="xs", bufs=2) as xpool,
            tc.tile_pool(name="stg", bufs=6) as spool,
            tc.tile_pool(name="dwa", bufs=4) as dpool,
            tc.tile_pool(name="pwb", bufs=3) as pwpool,
            tc.tile_pool(name="fw", bufs=4) as fwpool,
            tc.tile_pool(name="ob", bufs=10) as obpool,
            tc.tile_pool(name="ost", bufs=3) as opool,
            tc.tile_pool(name="ps_state", bufs=2, space="PSUM") as pspool,
            tc.tile_pool(name="ps_scr", bufs=2, space="PSUM") as scrpool,
        ):
            prew = cpool.tile([128, 2, 4, 128], F32R, tag="prew")
            dwt = cpool.tile([128, n["dve"], 50], F32, tag="dwt")
            alf = cpool.tile([128, n["conv"]], F32, tag="alf")
            alfc = cpool.tile([128, n["cheap"]], F32, tag="alfc")
            idx = cpool.tile([128, n["wave"]], I16, tag="idx")
            six = cpool.tile([128, n["slots"]], I16, tag="six")
            rmap = cpool.tile([128, 32, 32], F32, tag="rmap")
            for t, d in ((prew, prew_d), (dwt, dwt_d), (alf, alf_d),
                         (alfc, alfc_d), (idx, idx_d), (six, six_d),
                         (rmap, rmap_d)):
                nc.sync.dma_start(t[:], d)

            # state stack: 4 samples x 6 states, bf16
            states = wpool.tile([128, 24, 1024], BF16, tag="states")
            # per-step scatter target (separate tile so slot-output scatters
            # never alias the gathers reading `states`)
            newstate = wpool.tile([128, 4, 1024], BF16, tag="newstate")

            # pool scratch
            xpmax = wpool.tile([128, 34, 34], BF16, tag="xpmax")
            rmpad = wpool.tile([128, 34, 32], BF16, tag="rmpad")
            ptmp0 = wpool.tile([128, 32, 32], BF16, tag="ptmp0")
            xpsum = wpool.tile([128, 34, 34], F32, tag="xpsum")
            rspad = wpool.tile([128, 34, 32], F32, tag="rspad")
            ptmp1 = wpool.tile([128, 32, 32], F32, tag="ptmp1")
            nc.gpsimd.memset(xpmax[:], -1e30)
            nc.gpsimd.memset(rmpad[:], -1e30)
            nc.gpsimd.memset(xpsum[:], 0.0)
            nc.gpsimd.memset(rspad[:], 0.0)

            zbufs = [wpool.tile([128, 40, 40], BF16, tag=f"z{i}",
                                name=f"z{i}") for i in range(6)]
            for z in zbufs:
                nc.gpsimd.memset(z[:], 0.0)
            zctr = [0]

            def flat(ap3):
                return ap3.rearrange("p a b -> p (a b)")

            def relu_into_z(src_ap, scale):
                z = zbufs[zctr[0] % len(zbufs)]
                zctr[0] += 1
                nc.scalar.activation(z[:, 4:36, 4:36], src_ap, ACTF.Relu,
                                     scale=scale)
                return z

            def mm_chunks(psum3, lhsT, rhs3, flags):
                s0, e0, s1, e1 = flags
                nc.tensor.matmul(psum3[:, 0:16, :], lhsT, rhs3[:, 0:16, :],
                                 start=s0, stop=e0)
                nc.tensor.matmul(psum3[:, 16:32, :], lhsT, rhs3[:, 16:32, :],
                                 start=s1, stop=e1)

            def dw_chain(z, dslot, tap0, k, pad, stride):
                dwacc = dpool.tile([128, 32, 32], BF16, tag="dwacc")
                first = True
                for ky in range(k):
                    for kx in range(k):
                        t = tap0 + ky * k + kx
                        y0 = 4 - pad + stride * ky
                        x0 = 4 - pad + stride * kx
                        view = z[:, y0:y0 + 32, x0:x0 + 32]
                        sc = dwt[:, dslot, t:t + 1]
                        if first:
                            nc.vector.tensor_scalar_mul(dwacc[:], view, sc)
                            first = False
                        else:
                            nc.vector.scalar_tensor_tensor(
                                dwacc[:], view, sc, dwacc[:],
                                op0=ALU.mult, op1=ALU.add)
                return dwacc

            def fused_stage(stage_i, z, k, pad, stride, psum3):
                taps = k * k
                half = (taps + 1) // 2
                for (a, e) in ((0, half), (half, taps)):
                    fwt = fwpool.tile([128, 13, 128], BF16, tag="fw")
                    nc.sync.dma_start(fwt[:, 0:e - a, :],
                                      fw_d[:, stage_i, a:e, :])
                    for t in range(a, e):
                        ky, kx = divmod(t, k)
                        y0 = 4 - pad + stride * ky
                        x0 = 4 - pad + stride * kx
                        for h2 in range(2):
                            nc.tensor.matmul(
                                psum3[:, 16 * h2:16 * h2 + 16, :],
                                fwt[:, t - a, :],
                                z[:, y0 + 16 * h2:y0 + 16 * h2 + 16,
                                  x0:x0 + 32],
                                start=(t == 0), stop=(t == taps - 1))
                return psum3

            def stream_pw(pw_i):
                t = pwpool.tile([128, 1, 128], BF16, tag="pwb")
                nc.sync.dma_start(t[:], pw_d[:, pw_i:pw_i + 1, :])
                return t[:, 0, :]

            def conv_slot(o, x_ap, cs, ctr):
                """Emit conv slot; returns stp psum [128,32,32] result."""
                k, pad, stride = CONV_GEO[o]
                a_ap = alf[:, cs:cs + 1]
                stp = pspool.tile([128, 32, 32], F32, tag="stp")
                if forms[o] == "pe":
                    z1 = relu_into_z(x_ap, a_ap)
                    if o in TWO_STAGE:
                        scr = scrpool.tile([128, 32, 32], F32, tag="scr")
                        fused_stage(ctr["stage"], z1, k, pad, stride, scr)
                        ctr["stage"] += 1
                        z2 = relu_into_z(scr[:], 1.0)
                        fused_stage(ctr["stage"], z2, k, pad, stride, stp)
                        ctr["stage"] += 1
                    else:
                        fused_stage(ctr["stage"], z1, k, pad, stride, stp)
                        ctr["stage"] += 1
                else:
                    z1 = relu_into_z(x_ap, a_ap)
                    dwacc = dw_chain(z1, ctr["dve"], 0, k, pad, stride)
                    if o in TWO_STAGE:
                        scr = scrpool.tile([128, 32, 32], F32, tag="scr")
                        mm_chunks(scr, stream_pw(ctr["pw"]), dwacc,
                                  (True, True, True, True))
                        ctr["pw"] += 1
                        z2 = relu_into_z(scr[:], 1.0)
                        dwacc2 = dw_chain(z2, ctr["dve"], 25, k, pad, stride)
                        mm_chunks(stp, stream_pw(ctr["pw"]), dwacc2,
                                  (True, True, True, True))
                        ctr["pw"] += 1
                    else:
                        mm_chunks(stp, stream_pw(ctr["pw"]), dwacc,
                                  (True, True, True, True))
                        ctr["pw"] += 1
                    ctr["dve"] += 1
                return stp

            def cheap_slot(o, x_ap, cc_i, ob):
                """Compute cheap op into ob tile [128,32,32] (alpha-scaled)."""
                sc = alfc[:, cc_i:cc_i + 1]
                dst = ob[:]
                if o == O_SKIP:
                    nc.scalar.activation(dst, x_ap, ACTF.Copy, scale=sc)
                elif o == O_MAX:
                    nc.scalar.copy(xpmax[:, 1:33, 1:33], x_ap)
                    t = ptmp0
                    nc.vector.tensor_max(t[:], xpmax[:, 1:33, 0:32],
                                         xpmax[:, 1:33, 1:33])
                    nc.vector.tensor_max(rmpad[:, 1:33, :], t[:],
                                         xpmax[:, 1:33, 2:34])
                    nc.vector.tensor_max(t[:], rmpad[:, 0:32, :],
                                         rmpad[:, 1:33, :])
                    nc.vector.tensor_max(t[:], t[:], rmpad[:, 2:34, :])
                    nc.scalar.activation(dst, t[:], ACTF.Copy, scale=sc)
                else:  # O_AVG
                    eng = nc.gpsimd if avg_eng == "gpsimd" else nc.vector
                    nc.scalar.copy(xpsum[:, 1:33, 1:33], x_ap)
                    t = ptmp1
                    eng.tensor_add(t[:], xpsum[:, 1:33, 0:32],
                                   xpsum[:, 1:33, 1:33])
                    eng.tensor_add(rspad[:, 1:33, :], t[:],
                                   xpsum[:, 1:33, 2:34])
                    eng.tensor_add(t[:], rspad[:, 0:32, :],
                                   rspad[:, 1:33, :])
                    eng.tensor_add(t[:], t[:], rspad[:, 2:34, :])
                    eng.tensor_mul(t[:], t[:], rmap[:])
                    nc.scalar.activation(dst, t[:], ACTF.Copy, scale=sc)

            # ---- preprocess ----
            for bs in range(BL):
                for inp, xd in ((0, x0_d), (1, x1_d)):
                    scr = scrpool.tile([128, 32, 32], F32, tag="scr")
                    for kc in range(4):
                        xb = xpool.tile([128, HW], F32, tag="xb")
                        nc.sync.dma_start(xb[:], xd[bs, kc])
                        xr = xpool.tile([128, HW], F32R, tag="xr")
                        nc.scalar.activation(xr[:], xb[:], ACTF.Relu)
                        for h in range(2):
                            nc.tensor.matmul(
                                scr[:, 16 * h:16 * (h + 1), :],
                                prew[:, inp, kc, :],
                                xr[:, 512 * h:512 * (h + 1)].rearrange(
                                    "p (a c) -> p a c", a=16),
                                start=(kc == 0), stop=(kc == 3))
                    nc.scalar.copy(states[:, 6 * bs + inp, :].rearrange(
                        "p (h w) -> p h w", h=32), scr[:])

            # ---- steps ----
            ctr = dict(stage=0, pw=0, dve=0)
            n_slot_c = [0]
            n_conv_c = n_cheap_c = n_wave_c = 0
            for s in range(STEPS):
                convs, cheaps = sched[s]
                n_slots = len(convs) + len(cheaps)
                n_waves = (n_slots + 1) // 2
                stgs = {}

                def slot_x(i):
                    wv = i // 2
                    if wv not in stgs:
                        stg = spool.tile([128, 16, 128], BF16, tag="stg")
                        nc.gpsimd.ap_gather(
                            flat(stg[:]), flat(states[:]),
                            idx[:, n_wave_c + wv:n_wave_c + wv + 1],
                            channels=128, num_elems=192, d=128, num_idxs=16)
                        stgs[wv] = stg
                    stg = stgs[wv]
                    half = stg[:, 8 * (i % 2):8 * (i % 2) + 8, :]
                    return flat(half).rearrange("p (h w) -> p h w", h=32)

                nc.vector.memset(newstate[:], 0.0)

                def scatter(ob, si):
                    nc.gpsimd.scatter_add(
                        flat(newstate[:]).rearrange("p (a b) -> p a b", b=64),
                        six[:, si:si + 1],
                        flat(ob[:]).rearrange("p (a b) -> p a b", b=64),
                        channels=128, num_elems=64, d=64, num_idxs=16)

                # scatter of slot i is emitted after slot i+1's compute so
                # the (in-order) gpsimd queue isn't head-of-line blocked on
                # slot i's psum evac while later gathers wait behind it.
                pend = []

                def flush(keep):
                    while len(pend) > keep:
                        scatter(*pend.pop(0))

                for i, o in enumerate(convs):
                    stp = conv_slot(o, slot_x(i), n_conv_c + i, ctr)
                    ob = obpool.tile([128, 32, 32], BF16, tag="ob")
                    nc.scalar.copy(ob[:], stp[:])
                    pend.append((ob, n_slot_c[0] + i))
                    flush(3)
                for i, o in enumerate(cheaps):
                    ob = obpool.tile([128, 32, 32], BF16, tag="ob")
                    cheap_slot(o, slot_x(len(convs) + i), n_cheap_c + i, ob)
                    pend.append((ob, n_slot_c[0] + len(convs) + i))
                    flush(3)
                flush(0)
                n_conv_c += len(convs)
                n_cheap_c += len(cheaps)
                n_wave_c += n_waves
                n_slot_c[0] += n_slots

                for bs in range(BL):
                    nc.scalar.copy(states[:, 6 * bs + 2 + s, :],
                                   newstate[:, bs, :])
                    ost = opool.tile([128, 1024], F32, tag="ost")
                    nc.scalar.copy(ost[:], newstate[:, bs, :])
                    nc.sync.dma_start(out_d[bs, s], ost[:])

    nc.compile()
    return nc


# ---------------------------------------------------------------- host data

def host_prepare(inputs):
    s0, s1 = np.asarray(inputs["s0"]), np.asarray(inputs["s1"])
    gates = np.asarray(inputs["gates"])
    top = int(inputs["top"])
    plan = build_plan(gates, top)
    p, assign, sched, forms, n = (plan["p"], plan["assign"], plan["sched"],
                                  plan["forms"], plan["n"])

    prew = np.empty((128, 2, 4, 128), np.float32)
    for inp, wname in ((0, "pre0_w"), (1, "pre1_w")):
        wmat = np.asarray(inputs[wname]) * BN_SCALE
        for kc in range(4):
            prew[:, inp, kc, :] = wmat[:, 128 * kc:128 * (kc + 1)].T

    FUSE_KEYS = {O_SEP5: (("sep5_pw1", "sep5_dw1"), ("sep5_pw2", "sep5_dw2")),
                 O_DIL5: (("dil5_pw", "dil5_dw"),),
                 O_SEP3: (("sep3_pw1", "sep3_dw1"), ("sep3_pw2", "sep3_dw2")),
                 O_DIL3: (("dil3_pw", "dil3_dw"),)}

    def fuse(pw_key, dw_key, m, k):
        pwm = np.asarray(inputs[pw_key])[m].astype(np.float32) * BN_SCALE
        dwm = np.asarray(inputs[dw_key])[m].astype(np.float32).reshape(C, k * k)
        return pwm.T[:, None, :] * dwm[:, :, None]  # [ci, k*k, co]

    cnt = np.zeros((32, 32), np.float32)
    for dy in (-1, 0, 1):
        for dx in (-1, 0, 1):
            cnt[max(0, dy):32 - max(0, -dy),
                max(0, dx):32 - max(0, -dx)] += 1
    rmap = np.broadcast_to((BN_SCALE / cnt).astype(np.float32),
                           (128, 32, 32)).copy()

    act = plan["act"]
    in_maps = []
    for core in range(N_CORES):
        samples = assign[core]
        fw = np.zeros((128, n["stage"], 25, 128), np.float32)
        pw = np.zeros((128, n["pw"], 128), np.float32)
        dwt = np.zeros((128, n["dve"], 50), np.float32)
        alf_t = np.zeros((n["conv"],), np.float32)
        alfc_t = np.zeros((n["cheap"],), np.float32)
        idx_t = np.zeros((128, n["wave"]), np.int16)
        six_t = np.zeros((128, n["slots"]), np.int16)
        ns = dict(stage=0, pw=0, dve=0, conv=0, cheap=0, wave=0, slot=0)

        for s in range(STEPS):
            convs, cheaps = sched[s]
            items = {o: [] for o in CONV_OPS + CHEAP_OPS}
            for bs in range(BL):
                smp = samples[bs]
                for j in range(2 + s):
                    m = OFFSETS[s] + j
                    for o in CONV_OPS + CHEAP_OPS:
                        if act[m, smp, o]:
                            items[o].append((m, j, bs))
            used = {o: 0 for o in items}
            slot_src = []   # gather chunk base per slot (j-state of its bs)
            slot_tgt = []   # scatter target state index per slot
            for o in convs:
                if used[o] < len(items[o]):
                    m, j, bs = items[o][used[o]]
                    used[o] += 1
                    a = float(p[m, samples[bs], o])
                else:
                    m, j, bs, a = None, 0, 0, 0.0
                slot_src.append(6 * bs + j)
                slot_tgt.append(bs)
                alf_t[ns["conv"]] = a
                k, _, _ = CONV_GEO[o]
                if forms[o] == "pe":
                    for st_i, (pwk, dwk) in enumerate(FUSE_KEYS[o]):
                        if m is not None:
                            fw[:, ns["stage"], 0:k * k] = fuse(pwk, dwk, m, k)
                        ns["stage"] += 1
                else:
                    if m is not None:
                        for st_i, (pwk, dwk) in enumerate(FUSE_KEYS[o]):
                            dwm = np.asarray(inputs[dwk])[m].reshape(C, k * k)
                            dwt[:, ns["dve"], 25 * st_i:25 * st_i + k * k] = dwm
                            pw[:, ns["pw"] + st_i] = (
                                np.asarray(inputs[pwk])[m].T * BN_SCALE)
                    ns["pw"] += 2 if o in TWO_STAGE else 1
                    ns["dve"] += 1
                ns["conv"] += 1
            for o in cheaps:
                if used[o] < len(items[o]):
                    m, j, bs = items[o][used[o]]
                    used[o] += 1
                    a = float(p[m, samples[bs], o])
                else:
                    m, j, bs, a = None, 0, 0, 0.0
                slot_src.append(6 * bs + j)
                slot_tgt.append(bs)
                if o == O_MAX:
                    a *= BN_SCALE
                alfc_t[ns["cheap"]] = a
                ns["cheap"] += 1
            # gather idx: wave of 2 slots, chunks of 128 elems (8/state)
            for wv in range((len(slot_src) + 1) // 2):
                j1 = slot_src[2 * wv]
                j2 = slot_src[2 * wv + 1] if 2 * wv + 1 < len(slot_src) else 0
                vals = np.concatenate([8 * j1 + np.arange(8),
                                       8 * j2 + np.arange(8)])
                idx_t[:, ns["wave"]] = vals[np.arange(128) % 16]
                ns["wave"] += 1
            # scatter idx: 16 chunks of 64 elems at target state
            for t in slot_tgt:
                vals = 16 * t + np.arange(16)
                six_t[:, ns["slot"]] = vals[np.arange(128) % 16]
                ns["slot"] += 1

        import ml_dtypes
        in_maps.append({
            "x0": s0[samples].reshape(BL, 4, 128, HW).astype(np.float32),
            "x1": s1[samples].reshape(BL, 4, 128, HW).astype(np.float32),
            "prew": prew,
            "fw": fw.astype(ml_dtypes.bfloat16),
            "pw": pw.astype(ml_dtypes.bfloat16),
            "dwt": dwt,
            "alf": np.broadcast_to(alf_t, (128, n["conv"])).copy(),
            "alfc": np.broadcast_to(alfc_t, (128, n["cheap"])).copy(),
            "idx": idx_t, "six": six_t, "rmap": rmap,
        })
    return in_maps, plan


_prog_cache = {}


def _get_program(plan):
    key = plan["key"]
    if key not in _prog_cache:
        _prog_cache[key] = build_program(plan)
    return _prog_cache[key]


def prepare_run(inputs):
    in_maps, plan = host_prepare(inputs)
    return in_maps, _get_program(plan)


def kernel(**inputs):
    in_maps, plan = host_prepare(inputs)
    nc = _get_program(plan)
    res = run_bass_kernel_spmd(nc, in_maps, core_ids=list(range(N_CORES)))
    out = np.empty((B, 512, H, W), np.float32)
    for core in range(N_CORES):
        o = res.results[core]["out"]
        for bs in range(BL):
            out[plan["assign"][core][bs]] = (
                o[bs].reshape(512, H, W).astype(np.float32))
    return out
